# revision 22
# baseline (speedup 1.0000x reference)
"""GCN model (2x shared GCNConv+BN+LeakyReLU, linear head) on 8 trn2 NeuronCores.

Sharding: nodes row-sharded across 8 cores (12500 rows each). Small weights
replicated. Layer-1 conv input rows (z1 = x @ (W1@Wc)) are computed directly
for own + halo (1-hop in-neighbor) rows from host-staged transposed x slices
(no collective needed). Layer-2 conv inputs are exchanged with one AllToAll of
pre-BN aggregation rows; BN is applied by the receiver (global BN stats come
from a tiny AllReduce). Aggregation (segment-sum over edges) is done as
one-hot matmuls on the TensorEngine with dma_gather'ed rows as the moving
operand; self-loops are an identity-matrix chunk. The symmetric deg^-1/2
normalization is factorized: table rows are pre-scaled by dinv[src] and
aggregation outputs are post-scaled by dinv[dst], so the one-hot matrices are
exact 0/1 in bf16.

BN statistics ride the aggregation: per 128-feature chunk, one matmul
lhsT=agg_chunk, rhs=[agg_chunk | ones] accumulates the chunk Gram matrix and
the column sums in PSUM; sum-of-squares is the Gram diagonal (extracted with
affine_select), so no extra elementwise pass is needed.

Compute is bf16 with fp32 PSUM accumulation. Host-side work is integer graph
preprocessing / input staging; all FP math on node features runs on device.
"""

import math
import os

# If the host process pinned jax to cpu (common for running the reference),
# re-enable the axon platform the Bass runner needs.
if os.environ.get("AXON_LOOPBACK_RELAY") or os.environ.get("AXON_POOL_SVC_OVERRIDE"):
    _jp = os.environ.get("JAX_PLATFORMS")
    if _jp and "axon" not in _jp:
        os.environ["JAX_PLATFORMS"] = "axon," + _jp

import numpy as np
import ml_dtypes

from concourse import bacc, bass, mybir, tile
from concourse.bass_utils import run_bass_kernel_spmd

BF16 = mybir.dt.bfloat16
F32 = mybir.dt.float32
I16 = mybir.dt.int16
NP_BF16 = ml_dtypes.bfloat16

P = 128
EPS = 1e-5
ALPHA = 0.01
GRP = 4        # dst tiles per batched group
FILL_T = 8     # halo tiles per a2a-fill gather


def _wrap_idx(flat):
    """dma_gather index layout: idx i at [i % 16, i // 16], x8 across partitions."""
    n = len(flat)
    assert n % 16 == 0
    w = np.zeros((16, n // 16), np.int16)
    w[np.arange(n) % 16, np.arange(n) // 16] = flat.astype(np.int16)
    return np.ascontiguousarray(np.tile(w, (8, 1)))


# ---------------------------------------------------------------------------
# Host-side planning (graph preprocessing + input staging)
# ---------------------------------------------------------------------------

def make_plan(x, edge_index, W1, b1, Wc, bc, gamma, beta, W2, b2, WO, bO, C=8):
    x = np.asarray(x, np.float32)
    ei = np.asarray(edge_index).astype(np.int64)
    src, dst = ei[0], ei[1]
    N, F = x.shape
    H = np.asarray(Wc).shape[0]
    CH = H // P
    assert N % C == 0
    S = N // C
    T_OWN = math.ceil(S / P)
    OWN_PAD = T_OWN * P

    deg = np.bincount(dst, minlength=N).astype(np.float32) + np.float32(1.0)
    dinv = (np.float32(1.0) / np.sqrt(deg)).astype(np.float32)

    owner_s = src // S
    owner_d = dst // S

    U = [[None] * C for _ in range(C)]
    maxu = 1
    for j in range(C):
        for k in range(C):
            if j == k:
                continue
            m = (owner_s == j) & (owner_d == k)
            U[j][k] = np.unique(src[m])
            maxu = max(maxu, len(U[j][k]))
    R = math.ceil(maxu / P) * P
    SLOT_T = R // P
    SLOT_A = (SLOT_T + 1) // 2
    SLOT_B = SLOT_T - SLOT_A
    R_A, R_B = SLOT_A * P, SLOT_B * P
    T_HALO_A, T_HALO_B = C * SLOT_A, C * SLOT_B
    T_HALO = T_HALO_A + T_HALO_B
    HALO_ROWS = T_HALO * P
    RECVA = OWN_PAD                      # halo offsets within the table
    RECVB = OWN_PAD + T_HALO_A * P
    TABLE_ROWS = OWN_PAD + HALO_ROWS
    assert TABLE_ROWS <= 32767, "table must be addressable by int16 gather idxs"

    EC = 1
    core_edges = []
    for k in range(C):
        m = owner_d == k
        es, ed = src[m], dst[m] - k * S
        order = np.argsort(ed, kind="stable")
        es, ed = es[order], ed[order]
        tcnt = np.bincount(ed // P, minlength=T_OWN)
        if len(tcnt):
            EC = max(EC, math.ceil(tcnt.max() / P))
        core_edges.append((es, ed))

    W1 = np.asarray(W1, np.float64)
    Wc64 = np.asarray(Wc, np.float64)
    Wf = (W1 @ Wc64).astype(np.float32)
    bf_row = (np.asarray(b1, np.float64) @ Wc64).astype(np.float32)
    WfO = (np.asarray(W2, np.float64) @ np.asarray(WO, np.float64)).astype(np.float32)
    bOf = float(np.asarray(b2, np.float64) @ np.asarray(WO, np.float64)[:, 0]
                + np.asarray(bO, np.float64)[0])
    has_bias = bool(np.any(bf_row != 0.0))

    def _wlayout(W):  # [CH*P, H] -> [P, CH*H] with [p, c*H+n] = W[c*P+p, n]
        return np.ascontiguousarray(
            W.reshape(CH, P, H).transpose(1, 0, 2).reshape(P, CH * H).astype(NP_BF16))

    wf_host = _wlayout(Wf)
    wc_host = _wlayout(np.asarray(Wc, np.float32))
    wfo_host = np.ascontiguousarray(WfO.reshape(CH, P).T.astype(NP_BF16))
    gmb_host = np.concatenate(
        [np.asarray(gamma, np.float32).reshape(CH, P).T,
         np.asarray(beta, np.float32).reshape(CH, P).T], axis=1)
    eye_host = np.eye(P, dtype=NP_BF16)
    eye32_host = np.eye(P, dtype=np.float32)
    brow_host = bf_row.reshape(1, H).astype(NP_BF16)
    onesrow_host = np.ones((1, P), dtype=NP_BF16)

    in_maps = []
    for k in range(C):
        es, ed = core_edges[k]
        row = np.zeros(len(es), np.int64)
        own_mask = (es // S) == k
        row[own_mask] = es[own_mask] - k * S
        for j in range(C):
            if j == k:
                continue
            m = (es // S) == j
            if m.any():
                pos = np.searchsorted(U[j][k], es[m])
                row[m] = np.where(pos < R_A,
                                  RECVA + j * R_A + pos,
                                  RECVB + j * R_B + (pos - R_A))

        tile_id = ed // P
        starts = np.searchsorted(tile_id, np.arange(T_OWN))
        r = np.arange(len(ed)) - starts[tile_id]
        cch = r // P
        epos = r % P
        assert len(cch) == 0 or cch.max() < EC
        col = tile_id * EC + cch

        gflat = np.zeros(T_OWN * EC * P, np.int64)
        gflat[col * P + epos] = row
        gidx = _wrap_idx(gflat)

        oh = np.zeros((P, T_OWN * EC * P), NP_BF16)
        oh[epos, col * P + (ed % P)] = NP_BF16(1.0)

        ag_a = np.zeros(T_HALO_A * P, np.int64)
        ag_b = np.zeros(max(T_HALO_B * P, 16), np.int64)
        for j in range(C):
            if j == k:
                continue
            u = U[k][j]
            na = min(len(u), R_A)
            ag_a[j * R_A: j * R_A + na] = u[:na] - k * S
            if len(u) > R_A:
                ag_b[j * R_B: j * R_B + len(u) - R_A] = u[R_A:] - k * S
        agidx_a = _wrap_idx(ag_a)
        agidx_b = _wrap_idx(ag_b)

        xo = np.zeros((F, OWN_PAD), NP_BF16)
        xo[:, :S] = x[k * S:(k + 1) * S].T
        xh = np.zeros((F, HALO_ROWS), NP_BF16)
        dvh = np.zeros(HALO_ROWS, np.float32)
        for j in range(C):
            if j == k:
                continue
            u = U[j][k]
            na = min(len(u), R_A)
            oa = j * R_A
            xh[:, oa:oa + na] = x[u[:na]].T
            dvh[oa:oa + na] = dinv[u[:na]]
            if len(u) > R_A:
                ob = T_HALO_A * P + j * R_B
                xh[:, ob:ob + len(u) - R_A] = x[u[R_A:]].T
                dvh[ob:ob + len(u) - R_A] = dinv[u[R_A:]]

        dvo = np.zeros(OWN_PAD, np.float32)
        dvo[:S] = dinv[k * S:(k + 1) * S]
        dinv_own = np.ascontiguousarray(dvo.reshape(T_OWN, P).T)
        dinv_halo = np.ascontiguousarray(dvh.reshape(T_HALO, P).T)

        in_maps.append({
            "xt_own": xo, "xt_halo": xh, "onehot": oh, "gidx": gidx,
            "agidx_a": agidx_a, "agidx_b": agidx_b,
            "dinv_own": dinv_own, "dinv_halo": dinv_halo,
            "wf": wf_host, "wc": wc_host, "wfo": wfo_host, "gmb": gmb_host,
            "eye": eye_host, "eye32": eye32_host,
            "brow": brow_host, "onesrow": onesrow_host,
        })

    lo = 1
    for k in range(C):
        for j in range(C):
            if j == k:
                continue
            u = U[k][j]
            if len(u):
                na = min(len(u), R_A)
                lo = max(lo, int(u[na - 1] - k * S) + 1)
    LO_T = min(T_OWN, math.ceil(lo / P))

    dims = dict(N=N, H=H, CH=CH, C=C, S=S, T_OWN=T_OWN, OWN_PAD=OWN_PAD,
                EC=EC, R=R, T_HALO=T_HALO, T_HALO_A=T_HALO_A,
                T_HALO_B=T_HALO_B, HALO_ROWS=HALO_ROWS, LO_T=LO_T,
                TABLE_ROWS=TABLE_ROWS, bOf=bOf, has_bias=has_bias)
    return dims, in_maps


# ---------------------------------------------------------------------------
# Device program
# ---------------------------------------------------------------------------

def build_program(d):
    C, H, CH = d["C"], d["H"], d["CH"]
    S, T_OWN, OWN_PAD = d["S"], d["T_OWN"], d["OWN_PAD"]
    EC, T_HALO, HALO_ROWS = d["EC"], d["T_HALO"], d["HALO_ROWS"]
    T_HALO_A, T_HALO_B = d["T_HALO_A"], d["T_HALO_B"]
    TABLE_ROWS = d["TABLE_ROWS"]
    N = d["N"]
    CW = CH * P + CH            # per-tile agg row width incl. ones columns
    groups = [list(range(C))]
    Lrelu = mybir.ActivationFunctionType.Lrelu
    Sqrt = mybir.ActivationFunctionType.Sqrt
    Sigmoid = mybir.ActivationFunctionType.Sigmoid
    AxX = mybir.AxisListType.X
    Add = mybir.AluOpType.add

    nc = bacc.Bacc("TRN2", target_bir_lowering=False, debug=False,
                   enable_asserts=False, num_devices=C)

    xt_own_d = nc.dram_tensor("xt_own", [H, OWN_PAD], BF16, kind="ExternalInput")
    xt_halo_d = nc.dram_tensor("xt_halo", [H, HALO_ROWS], BF16, kind="ExternalInput")
    onehot_d = nc.dram_tensor("onehot", [P, T_OWN * EC * P], BF16, kind="ExternalInput")
    gidx_d = nc.dram_tensor("gidx", [P, T_OWN * EC * P // 16], I16, kind="ExternalInput")
    agidx_a_d = nc.dram_tensor("agidx_a", [P, T_HALO_A * P // 16], I16,
                               kind="ExternalInput")
    agidx_b_d = nc.dram_tensor("agidx_b", [P, max(T_HALO_B * P, 16) // 16], I16,
                               kind="ExternalInput")
    dinv_own_d = nc.dram_tensor("dinv_own", [P, T_OWN], F32, kind="ExternalInput")
    dinv_halo_d = nc.dram_tensor("dinv_halo", [P, T_HALO], F32, kind="ExternalInput")
    wf_d = nc.dram_tensor("wf", [P, CH * H], BF16, kind="ExternalInput")
    wc_d = nc.dram_tensor("wc", [P, CH * H], BF16, kind="ExternalInput")
    wfo_d = nc.dram_tensor("wfo", [P, CH], BF16, kind="ExternalInput")
    gmb_d = nc.dram_tensor("gmb", [P, 2 * CH], F32, kind="ExternalInput")
    eye_d = nc.dram_tensor("eye", [P, P], BF16, kind="ExternalInput")
    eye32_d = nc.dram_tensor("eye32", [P, P], F32, kind="ExternalInput")
    brow_d = nc.dram_tensor("brow", [1, H], BF16, kind="ExternalInput")
    onesrow_d = nc.dram_tensor("onesrow", [1, P], BF16, kind="ExternalInput")
    out_ext = nc.dram_tensor("out", [S, 1], F32, kind="ExternalOutput")

    def cdiv(a, b):
        return (a + b - 1) // b

    with tile.TileContext(nc) as tc:
        with (
            tc.tile_pool(name="consts", bufs=1) as cp,
            tc.tile_pool(name="resid", bufs=1) as rp,
            tc.tile_pool(name="work", bufs=2) as wp,
            tc.tile_pool(name="psum", bufs=1, space="PSUM") as pp,
            tc.tile_pool(name="dram", bufs=1, space="DRAM") as dp,
        ):
            # ---- constants
            gidx_sb = cp.tile([P, T_OWN * EC * P // 16], I16, name="gidx_sb")
            nc.sync.dma_start(out=gidx_sb, in_=gidx_d[:, :])
            agidx_a_sb = cp.tile([P, T_HALO_A * P // 16], I16, name="agidx_a_sb")
            nc.sync.dma_start(out=agidx_a_sb, in_=agidx_a_d[:, :])
            agidx_b_sb = cp.tile([P, max(T_HALO_B * P, 16) // 16], I16,
                                 name="agidx_b_sb")
            nc.sync.dma_start(out=agidx_b_sb, in_=agidx_b_d[:, :])
            dvo_sb = cp.tile([P, T_OWN], F32, name="dvo_sb")
            nc.sync.dma_start(out=dvo_sb, in_=dinv_own_d[:, :])
            dvh_sb = cp.tile([P, T_HALO], F32, name="dvh_sb")
            nc.sync.dma_start(out=dvh_sb, in_=dinv_halo_d[:, :])
            wf_sb = cp.tile([P, CH * H], BF16, name="wf_sb")
            nc.sync.dma_start(out=wf_sb, in_=wf_d[:, :])
            wc_sb = cp.tile([P, CH * H], BF16, name="wc_sb")
            nc.sync.dma_start(out=wc_sb, in_=wc_d[:, :])
            wfo_sb = cp.tile([P, CH], BF16, name="wfo_sb")
            nc.sync.dma_start(out=wfo_sb, in_=wfo_d[:, :])
            gmb_sb = cp.tile([P, 2 * CH], F32, name="gmb_sb")
            nc.sync.dma_start(out=gmb_sb, in_=gmb_d[:, :])
            eye_sb = cp.tile([P, P], BF16, name="eye_sb")
            nc.sync.dma_start(out=eye_sb, in_=eye_d[:, :])
            eye32_sb = cp.tile([P, P], F32, name="eye32_sb")
            nc.sync.dma_start(out=eye32_sb, in_=eye32_d[:, :])
            brow_sb = cp.tile([1, H], BF16, name="brow_sb")
            nc.sync.dma_start(out=brow_sb, in_=brow_d[:, :])
            onesrow_sb = cp.tile([1, P], BF16, name="onesrow_sb")
            nc.sync.dma_start(out=onesrow_sb, in_=onesrow_d[:, :])
            zrow_sb = cp.tile([1, 2 * (P + 1)], BF16, name="zrow_sb")
            nc.vector.memset(zrow_sb, 0.0)

            # ---- DRAM internals
            table1 = dp.tile([TABLE_ROWS, H], BF16, name="table1")
            table2 = dp.tile([TABLE_ROWS, H], BF16, name="table2")
            agg1 = dp.tile([OWN_PAD, H], BF16, name="agg1")
            a2a_in_a = dp.tile([T_HALO_A * P, H], BF16, name="a2a_in_a")
            recv2_a = dp.tile([T_HALO_A * P, H], BF16, name="recv2_a")
            if T_HALO_B:
                a2a_in_b = dp.tile([T_HALO_B * P, H], BF16, name="a2a_in_b")
                recv2_b = dp.tile([T_HALO_B * P, H], BF16, name="recv2_b")
            ar_in1 = dp.tile([P, 2 * CH], F32, name="ar_in1")
            ar_out1 = dp.tile([P, 2 * CH], F32, addr_space="Shared", name="ar_out1")
            ar_in2 = dp.tile([P, 2 * CH], F32, name="ar_in2")
            ar_out2 = dp.tile([P, 2 * CH], F32, addr_space="Shared", name="ar_out2")

            # ---- stages
            def z_from_x(xt_d, n_tiles, dv_sb, dest, dest_off, w_sb, tagp):
                for g0 in range(0, n_tiles, GRP):
                    gn = min(GRP, n_tiles - g0)
                    xt4 = wp.tile([P, gn * H], BF16, tag="xt4", bufs=3,
                                  name=f"xt4_{tagp}_{g0}")
                    nc.sync.dma_start(
                        out=xt4.rearrange("p (c q n) -> p c q n", q=gn, n=P),
                        in_=xt_d[:, g0 * P:(g0 + gn) * P].rearrange(
                            "(c p) (q n) -> p c q n", p=P, n=P))
                    zh4 = wp.tile([P, gn * H], BF16, tag="zh4", bufs=3,
                                  name=f"zh4_{tagp}_{g0}")
                    for q in range(gn):
                        t = g0 + q
                        zp = pp.tile([P, H], F32, tag="big", bufs=6,
                                     name=f"zp_{tagp}_{t}")
                        for c in range(CH):
                            last = (c == CH - 1) and not d["has_bias"]
                            nc.tensor.matmul(
                                zp, lhsT=xt4[:, (c * gn + q) * P:(c * gn + q + 1) * P],
                                rhs=w_sb[:, c * H:(c + 1) * H],
                                start=(c == 0), stop=last)
                        if d["has_bias"]:
                            nc.tensor.matmul(zp, lhsT=onesrow_sb[:, :],
                                             rhs=brow_sb[:, :], start=False,
                                             stop=True)
                        nc.vector.tensor_scalar_mul(
                            zh4[:, q * H:(q + 1) * H], zp, dv_sb[:, t:t + 1])
                    nc.sync.dma_start(
                        out=dest[dest_off + g0 * P: dest_off + (g0 + gn) * P,
                                 :].rearrange("(q p) h -> p q h", p=P),
                        in_=zh4.rearrange("p (q h) -> p q h", h=H))

            def agg_stage(table, lname, aggd=None, mid_group=None, mid_cb=None):
                """Aggregate into per-group resident tiles (width CW incl.
                ones cols); Gram+sum stats accumulate in PSUM. Returns
                (list of (group tile, gn), gram psum tiles)."""
                gram_a = pp.tile([P, 2 * (P + 1)], F32, tag="gram", bufs=2,
                                 name=f"gram_a_{lname}")
                gram_b = pp.tile([P, 2 * (P + 1)], F32, tag="gram", bufs=2,
                                 name=f"gram_b_{lname}")
                nc.tensor.matmul(gram_a, lhsT=onesrow_sb[:, :], rhs=zrow_sb,
                                 start=True, stop=False, skip_group_check=True)
                nc.tensor.matmul(gram_b, lhsT=onesrow_sb[:, :], rhs=zrow_sb,
                                 start=True, stop=False, skip_group_check=True)
                gram = [gram_a, gram_b]
                res = []
                for g0 in range(0, T_OWN, GRP):
                    gn = min(GRP, T_OWN - g0)
                    zt4 = wp.tile([P, gn * H], BF16, tag="zt4", bufs=3,
                                  name=f"zt4_{lname}_{g0}")
                    nc.sync.dma_start(
                        out=zt4.rearrange("p (q h) -> p q h", h=H),
                        in_=table[g0 * P:(g0 + gn) * P, :].rearrange(
                            "(q p) h -> p q h", p=P))
                    oh4 = wp.tile([P, gn * EC * P], BF16, tag="oh4", bufs=3,
                                  name=f"oh4_{lname}_{g0}")
                    nc.sync.dma_start(
                        out=oh4, in_=onehot_d[:, g0 * EC * P:(g0 + gn) * EC * P])
                    ni = gn * EC * P
                    g4 = wp.tile([P, gn * EC * H], BF16, tag="g4", bufs=3,
                                 name=f"g4_{lname}_{g0}")
                    # dma_gather caps at 1024 indices per call
                    nb = gn * EC
                    for b0 in range(0, nb, 8):
                        bn_ = min(8, nb - b0)
                        sni = bn_ * P
                        i0 = (g0 * EC + b0) * P // 16
                        nc.gpsimd.dma_gather(
                            out_ap=g4[:, b0 * H:(b0 + bn_) * H].rearrange(
                                "p (b h) -> p b h", h=H),
                            in_ap=table[:, :],
                            idxs_ap=gidx_sb[:, i0:i0 + sni // 16],
                            num_idxs=sni, num_idxs_reg=sni, elem_size=H)
                    asc4 = rp.tile([P, gn * CW], BF16, tag="ascres", bufs=cdiv(T_OWN, GRP),
                                   name=f"asc4_{lname}_{g0}")
                    nc.vector.memset(
                        asc4.rearrange("p (q c n) -> p q c n", c=CH,
                                       n=P + 1)[:, :, :, P:P + 1], 1.0)
                    for q in range(gn):
                        t = g0 + q
                        aps = pp.tile([P, H], F32, tag="big", bufs=6,
                                      name=f"aggp_{lname}_{t}")
                        nc.tensor.matmul(aps, lhsT=eye_sb,
                                         rhs=zt4[:, q * H:(q + 1) * H],
                                         start=True, stop=False,
                                         skip_group_check=True)
                        for c in range(EC):
                            b = q * EC + c
                            nc.tensor.matmul(aps, lhsT=oh4[:, b * P:(b + 1) * P],
                                             rhs=g4[:, b * H:(b + 1) * H],
                                             start=False, stop=(c == EC - 1),
                                             skip_group_check=True)
                        nc.vector.tensor_scalar_mul(
                            asc4.rearrange("p (q c n) -> p q c n", c=CH,
                                           n=P + 1)[:, q, :, 0:P],
                            aps.rearrange("p (c n) -> p c n", n=P),
                            dvo_sb[:, t:t + 1])
                        last = (g0 + gn >= T_OWN) and (q == gn - 1)
                        for c in (range(CH) if not os.environ.get("K_NO_GRAM") else []):
                            o = q * CW + c * (P + 1)
                            nc.tensor.matmul(
                                gram[c // 2][:, (c % 2) * (P + 1):
                                             (c % 2 + 1) * (P + 1)],
                                lhsT=asc4[:, o:o + P],
                                rhs=asc4[:, o:o + P + 1],
                                start=False, stop=last, skip_group_check=True)
                    if aggd is not None:
                        for q in range(gn):
                            t = g0 // 1 + q  # g0 is row-group start
                            nc.sync.dma_start(
                                out=aggd[g0 * P + q * P:
                                         g0 * P + (q + 1) * P, :],
                                in_=asc4.rearrange(
                                    "p (q c n) -> p q c n", c=CH,
                                    n=P + 1)[:, q, :, 0:P])
                    res.append((asc4, gn))
                    if mid_cb is not None and g0 // GRP == mid_group:
                        mid_cb()
                return res, gram

            def bn_coeffs(gram, ar_in, ar_out, lname):
                ssb = cp.tile([P, 2 * CH], F32, name=f"ssb_{lname}")
                for c in range(CH):
                    gsb = cp.tile([P, P + 1], F32, name=f"gsb_{lname}_{c}")
                    nc.vector.tensor_copy(
                        gsb, gram[c // 2][:, (c % 2) * (P + 1):
                                          (c % 2 + 1) * (P + 1)])
                    nc.vector.tensor_copy(ssb[:, c:c + 1], gsb[:, P:P + 1])
                    dsel = cp.tile([P, P], F32, name=f"dsel_{lname}_{c}")
                    nc.vector.tensor_mul(dsel, gsb[:, 0:P], eye32_sb)
                    nc.vector.tensor_reduce(ssb[:, CH + c:CH + c + 1], dsel,
                                            axis=AxX, op=Add)
                nc.sync.dma_start(out=ar_in[:, :], in_=ssb)
                nc.gpsimd.collective_compute(
                    "AllReduce", Add, replica_groups=groups,
                    ins=[ar_in.opt()], outs=[ar_out.opt()])
                sg = cp.tile([P, 2 * CH], F32, name=f"sg_{lname}")
                nc.sync.dma_start(out=sg, in_=ar_out[:, :])
                mu = cp.tile([P, CH], F32, name=f"mu_{lname}")
                nc.vector.tensor_scalar_mul(mu, sg[:, 0:CH], 1.0 / N)
                ex2 = cp.tile([P, CH], F32, name=f"ex2_{lname}")
                nc.vector.tensor_scalar_mul(ex2, sg[:, CH:2 * CH], 1.0 / N)
                var = cp.tile([P, CH], F32, name=f"var_{lname}")
                nc.vector.tensor_mul(var, mu, mu)
                nc.vector.tensor_sub(var, ex2, var)
                nc.vector.tensor_scalar_add(var, var, EPS)
                std = cp.tile([P, CH], F32, name=f"std_{lname}")
                nc.scalar.activation(std, var, Sqrt)
                rstd = cp.tile([P, CH], F32, name=f"rstd_{lname}")
                nc.vector.reciprocal(rstd, std)
                scale = cp.tile([P, CH], F32, name=f"scale_{lname}")
                nc.vector.tensor_mul(scale, gmb_sb[:, 0:CH], rstd)
                shift = cp.tile([P, CH], F32, name=f"shift_{lname}")
                nc.vector.tensor_mul(shift, mu, scale)
                nc.vector.tensor_sub(shift, gmb_sb[:, 2 * CH - CH + 0:2 * CH], shift)
                return scale, shift

            def bn_transpose_pair(src_sb, off, qn, scale, shift, tagp, t):
                """PE-transpose CH chunks of qn (1-2) node-major row-tiles
                into one PSUM bank, then one BN+LReLU ACT per chunk covering
                all qn tiles. Returns hts[c] = [P, qn*P] bf16 tile."""
                trp = pp.tile([P, CH * qn * P], BF16, tag="big", bufs=6,
                              name=f"trp_{tagp}_{t}")
                for c in range(CH):
                    for dq in range(qn):
                        nc.tensor.matmul(
                            trp[:, (c * qn + dq) * P:(c * qn + dq + 1) * P],
                            lhsT=src_sb[:, off(c, dq):off(c, dq) + P],
                            rhs=eye_sb, is_transpose=True,
                            start=(c == 0 and dq == 0),
                            stop=(c == CH - 1 and dq == qn - 1),
                            skip_group_check=True)
                hts = []
                for c in range(CH):
                    ht = wp.tile([P, qn * P], BF16, tag="h2t", bufs=8,
                                 name=f"h2t_{tagp}_{t}_{c}")
                    nc.scalar.activation(
                        ht, trp[:, c * qn * P:(c + 1) * qn * P], Lrelu,
                        bias=shift[:, c:c + 1], scale=scale[:, c:c + 1],
                        alpha=ALPHA)
                    hts.append(ht)
                return hts

            def conv_own(res, dv_sb, scale, shift, dest, w_sb, tagp):
                for gi, (asc4, gn) in enumerate(res):
                    g0 = gi * GRP
                    zh4 = wp.tile([P, gn * H], BF16, tag="zh4", bufs=3,
                                  name=f"zh4_{tagp}_{g0}")
                    for q0 in range(0, gn, 2):
                        qn = min(2, gn - q0)
                        t = g0 + q0
                        hts = bn_transpose_pair(
                            asc4,
                            lambda c, dq, q0=q0: (q0 + dq) * CW + c * (P + 1),
                            qn, scale, shift, tagp, t)
                        for dq in range(qn):
                            q = q0 + dq
                            zp = pp.tile([P, H], F32, tag="big", bufs=6,
                                         name=f"zp2_{tagp}_{g0}_{q}")
                            for c in range(CH):
                                nc.tensor.matmul(
                                    zp, lhsT=hts[c][:, dq * P:(dq + 1) * P],
                                    rhs=w_sb[:, c * H:(c + 1) * H],
                                    start=(c == 0), stop=(c == CH - 1))
                            nc.vector.tensor_scalar_mul(
                                zh4[:, q * H:(q + 1) * H], zp,
                                dv_sb[:, g0 + q:g0 + q + 1])
                    nc.sync.dma_start(
                        out=dest[g0 * P:(g0 + gn) * P, :].rearrange(
                            "(q p) h -> p q h", p=P),
                        in_=zh4.rearrange("p (q h) -> p q h", h=H))

            def conv_halo(src_d, n_tiles, dv_sb, dv_off, scale, shift, dest,
                          dest_off, w_sb, tagp):
                for g0 in range(0, n_tiles, GRP):
                    gn = min(GRP, n_tiles - g0)
                    r4 = wp.tile([P, gn * H], BF16, tag="zt4", bufs=3,
                                 name=f"r4_{tagp}_{g0}")
                    nc.sync.dma_start(
                        out=r4.rearrange("p (q h) -> p q h", h=H),
                        in_=src_d[g0 * P:(g0 + gn) * P, :].rearrange(
                            "(q p) h -> p q h", p=P))
                    zh4 = wp.tile([P, gn * H], BF16, tag="zh4", bufs=3,
                                  name=f"zh4_{tagp}_{g0}")
                    for q0 in range(0, gn, 2):
                        qn = min(2, gn - q0)
                        t = g0 + q0
                        hts = bn_transpose_pair(
                            r4, lambda c, dq, q0=q0: (q0 + dq) * H + c * P,
                            qn, scale, shift, tagp, t)
                        for dq in range(qn):
                            q = q0 + dq
                            zp = pp.tile([P, H], F32, tag="big", bufs=6,
                                         name=f"zp2_{tagp}_{g0}_{q}")
                            for c in range(CH):
                                nc.tensor.matmul(
                                    zp, lhsT=hts[c][:, dq * P:(dq + 1) * P],
                                    rhs=w_sb[:, c * H:(c + 1) * H],
                                    start=(c == 0), stop=(c == CH - 1))
                            nc.vector.tensor_scalar_mul(
                                zh4[:, q * H:(q + 1) * H], zp,
                                dv_sb[:, dv_off + g0 + q:dv_off + g0 + q + 1])
                    nc.sync.dma_start(
                        out=dest[dest_off + g0 * P: dest_off + (g0 + gn) * P,
                                 :].rearrange("(q p) h -> p q h", p=P),
                        in_=zh4.rearrange("p (q h) -> p q h", h=H))

            def fill(a2a_in, aidx_sb, n_tiles, tagp, src_rows):
                for g0 in range(0, n_tiles, FILL_T):
                    fn = min(FILL_T, n_tiles - g0)
                    ni = fn * P
                    gg = wp.tile([P, fn * H], BF16, tag="g4", bufs=3,
                                 name=f"gg_{tagp}_{g0}")
                    nc.gpsimd.dma_gather(
                        out_ap=gg.rearrange("p (b h) -> p b h", h=H),
                        in_ap=agg1[0:src_rows, :],
                        idxs_ap=aidx_sb[:, g0 * P // 16:(g0 + fn) * P // 16],
                        num_idxs=ni, num_idxs_reg=ni, elem_size=H)
                    nc.sync.dma_start(
                        out=a2a_in[g0 * P:(g0 + fn) * P, :].rearrange(
                            "(b p) h -> p b h", p=P),
                        in_=gg.rearrange("p (b h) -> p b h", h=H))

            # ================= pipeline =================
            STOP = int(os.environ.get("K_STOP_AFTER", "99"))
            z_from_x(xt_own_d, T_OWN, dvo_sb, table1, 0, wf_sb, "zo")
            z_from_x(xt_halo_d, T_HALO, dvh_sb, table1, OWN_PAD, wf_sb, "zhh")

            def _fill_a():
                fill(a2a_in_a, agidx_a_sb, T_HALO_A, "fa", d["LO_T"] * P)

            mid = min((d["LO_T"] + GRP - 1) // GRP, max(T_OWN // GRP - 1, 0))
            res1, gram1 = (agg_stage(table1, "l1", aggd=agg1, mid_group=mid,
                                     mid_cb=_fill_a)
                           if STOP >= 2 else (None, None))
            if STOP >= 4:
                scale1, shift1 = bn_coeffs(gram1, ar_in1, ar_out1, "l1")

            # a2a fill happens via fill() defined above
            if STOP >= 5:
                if T_HALO_B:
                    fill(a2a_in_b, agidx_b_sb, T_HALO_B, "fb", OWN_PAD)
                nc.gpsimd.collective_compute(
                    "AllToAll", mybir.AluOpType.bypass, replica_groups=groups,
                    ins=[a2a_in_a.opt()], outs=[recv2_a.opt()])
                if T_HALO_B:
                    nc.gpsimd.collective_compute(
                        "AllToAll", mybir.AluOpType.bypass,
                        replica_groups=groups,
                        ins=[a2a_in_b.opt()], outs=[recv2_b.opt()])

            if STOP >= 6:
                conv_own(res1, dvo_sb, scale1, shift1, table2, wc_sb, "co")
            if STOP >= 7:
                conv_halo(recv2_a, T_HALO_A, dvh_sb, 0, scale1, shift1,
                          table2, OWN_PAD, wc_sb, "cha")
            if STOP >= 7 and T_HALO_B:
                conv_halo(recv2_b, T_HALO_B, dvh_sb, T_HALO_A, scale1, shift1,
                          table2, OWN_PAD + T_HALO_A * P, wc_sb, "chb")

            res2, gram2 = agg_stage(table2, "l2") if STOP >= 8 else (None, None)
            if STOP >= 9:
                scale2, shift2 = bn_coeffs(gram2, ar_in2, ar_out2, "l2")

            # output head
            outcols = cp.tile([P, P], F32, name="outcols")
            nc.vector.memset(outcols, 0.0)
            for gi, (asc4, gn) in enumerate(res2 if STOP >= 10 else []):
                g0 = gi * GRP
                for q0 in range(0, gn, 2):
                    qn = min(2, gn - q0)
                    hts = bn_transpose_pair(
                        asc4, lambda c, dq, q0=q0: (q0 + dq) * CW + c * (P + 1),
                        qn, scale2, shift2, "o", g0 + q0)
                    for dq in range(qn):
                        t = g0 + q0 + dq
                        op = pp.tile([P, 1], F32, tag="gram", bufs=2,
                                     name=f"op_{t}")
                        for c in range(CH):
                            nc.tensor.matmul(
                                op, lhsT=hts[c][:, dq * P:(dq + 1) * P],
                                rhs=wfo_sb[:, c:c + 1],
                                start=(c == 0), stop=(c == CH - 1),
                                skip_group_check=True)
                        nc.vector.tensor_copy(outcols[:, t:t + 1], op)

            tpsum = pp.tile([P, P], F32, tag="big", bufs=6, name="tpsum")
            nc.tensor.transpose(out=tpsum, in_=outcols, identity=eye32_sb)
            sig = cp.tile([P, P], F32, name="sig")
            nc.scalar.activation(sig, tpsum, Sigmoid, bias=float(d["bOf"]),
                                 scale=1.0)
            full_t = S // P
            rem = S - full_t * P
            if full_t:
                nc.sync.dma_start(
                    out=out_ext[0:full_t * P, :].rearrange(
                        "(t p) o -> t (p o)", p=P),
                    in_=sig[0:full_t, :])
            if rem:
                nc.sync.dma_start(
                    out=out_ext[full_t * P:S, :].rearrange(
                        "(o p) one -> o (p one)", o=1),
                    in_=sig[full_t:full_t + 1, 0:rem])

    nc.compile()
    return nc


# ---------------------------------------------------------------------------
# Entry point
# ---------------------------------------------------------------------------

_CACHE = {}


def _get_program(dims):
    key = tuple(sorted((k, v) for k, v in dims.items()))
    if key not in _CACHE:
        _CACHE[key] = build_program(dims)
    return _CACHE[key]


def kernel(x, edge_index, W1, b1, Wc, bc, gamma, beta, W2, b2, WO, bO,
           trace=False):
    dims, in_maps = make_plan(x, edge_index, W1, b1, Wc, bc, gamma, beta,
                              W2, b2, WO, bO)
    nc = _get_program(dims)
    res = run_bass_kernel_spmd(nc, in_maps, core_ids=list(range(dims["C"])),
                               trace=trace)
    out = np.concatenate([r["out"] for r in res.results], axis=0)
    kernel.last_results = res
    return out.astype(np.float32)
